# revision 2
# baseline (speedup 1.0000x reference)
"""Trainium2 Bass kernel for the LayerNorm-RNN — time-parallel windowed scan.

Math (per batch b): u_t = (x_t @ W_e2s + b) @ Bm;  z_t = s_{t-1}@A + u_t;
s_t = LN(z_t)*gamma + beta;  out_t = (s_t @ C) @ W_s2o + b_s2o.

Folds (host): W_u = W_e2s@Bm, b_u = b_e2s@Bm + beta@A, G = diag(gamma)@A,
W_o = (diag(gamma)@C)@W_s2o, b_out = beta@C@W_s2o + b_s2o, cneg = -beta@A.
Recurrence on pre-norm z with centered state w = z - mu*1:
    v = w @ G          (since (z-mu*1)@G = z@G - mu*grow)
    z' = rr*v + u'_{t+1},   whitened state t = rr*w.

KEY IDEAS
1. The LN recurrence is strongly contracting (~0.66x/step): a scan from
   zero state converges to the true trajectory in ~16 steps.  Each core
   handles a 256-step TIME window of the sequence for ALL 8 batch
   elements; windows are spliced with 16 warmup steps.
2. The window is split into 6 sub-windows (~43 steps each), organized
   as 2 interleaved GANGS of 3 chains.  A gang advances its 3 chains in
   lockstep with a (k, chain, batch) column layout, so each of the 16
   G-tile matmuls per step covers all 3 chains with one 24-wide rhs
   (one weight load per tile per gang-step), and all elementwise ops
   are single wide instructions.  The two gangs interleave to hide the
   serial stats round-trip (PE->DVE->ACT->DVE) latency.
3. Core 0 needs the exact zero init at t=0: host passes zinit/wmask
   that splice z_0 = u'_0 + cneg at the warmup boundary of gang A
   chain 0.
"""

import sys
import os
from contextlib import ExitStack

import numpy as np

for _p in ("/opt/trn_rl_repo",):
    if _p not in sys.path and os.path.isdir(_p):
        sys.path.insert(0, _p)

B, T, E, S = 8, 2048, 1024, 512
LN_EPS = 1e-5
NCORES = 8
WOUT = 256           # output steps per core
WARM = 16            # warmup steps per chain
BASES = ((0, 86, 172), (43, 129, 215))      # gang A / gang B chain bases (rel)
WLEN = ((43, 43, 43), (43, 43, 41))         # output steps per chain
CS = tuple(tuple(WARM + w for w in ws) for ws in WLEN)   # (59,59,59),(59,59,57)
NW = WARM + WOUT     # 272 u' entries per core

_CACHE = {}


def build():
    import concourse.bass as bass
    import concourse.bacc as bacc
    from concourse import mybir
    from concourse.tile import TileContext

    f32 = mybir.dt.float32
    bf16 = mybir.dt.bfloat16
    AF = mybir.ActivationFunctionType
    ALU = mybir.AluOpType
    AP = bass.AP

    nc = bacc.Bacc(trn_type="TRN2")

    xt = nc.dram_tensor("xt", [B, 8, 128, NW], bf16, kind="ExternalInput")
    wu = nc.dram_tensor("wu", [8, 4, 128, 128], bf16, kind="ExternalInput")
    gt = nc.dram_tensor("gt", [4, 4, 128, 128], bf16, kind="ExternalInput")
    wo = nc.dram_tensor("wo", [S, E], bf16, kind="ExternalInput")
    bud = nc.dram_tensor("buc", [128, 4], f32, kind="ExternalInput")
    cnegd = nc.dram_tensor("cneg", [128, 4], f32, kind="ExternalInput")
    bod = nc.dram_tensor("bo", [1, E], f32, kind="ExternalInput")
    bo16d = nc.dram_tensor("bo16", [1, E], mybir.dt.bfloat16, kind="ExternalInput")
    zinitd = nc.dram_tensor("zinit", [128, 32], f32, kind="ExternalInput")
    wmaskd = nc.dram_tensor("wmask", [128, 32], f32, kind="ExternalInput")
    y = nc.dram_tensor("y", [B, WOUT, E], f32, kind="ExternalOutput")

    with ExitStack() as ctx:
        tc = ctx.enter_context(TileContext(nc))
        singles = ctx.enter_context(tc.tile_pool(name="singles", bufs=1))
        psum_big = ctx.enter_context(tc.tile_pool(name="psum_big", bufs=2, space="PSUM"))
        psum_w = ctx.enter_context(tc.tile_pool(name="psum_w", bufs=3, space="PSUM"))
        psum_ab = ctx.enter_context(tc.tile_pool(name="psum_ab", bufs=3, space="PSUM"))
        opool = ctx.enter_context(tc.tile_pool(name="opool", bufs=2))

        # ---- resident weights / constants ----
        wu_sb = singles.tile([128, 8, 4, 128], bf16)
        nc.sync.dma_start(out=wu_sb, in_=wu.rearrange("k m p q -> p k m q"))
        gt_sb = singles.tile([128, 4, 4, 128], bf16)
        nc.sync.dma_start(out=gt_sb, in_=gt.rearrange("k m p q -> p k m q"))
        wo_sb = singles.tile([128, 4, E], bf16)
        bu_sb = singles.tile([128, 4], f32)
        nc.sync.dma_start(out=bu_sb, in_=bud[:])
        cneg_sb = singles.tile([128, 4], f32)
        nc.sync.dma_start(out=cneg_sb, in_=cnegd[:])
        zinit_sb = singles.tile([128, 32], f32)
        nc.sync.dma_start(out=zinit_sb, in_=zinitd[:])
        wmask_sb = singles.tile([128, 32], f32)
        nc.sync.dma_start(out=wmask_sb, in_=wmaskd[:])
        bo_ap = bod[:]
        bo_sb = singles.tile([128, E], f32)
        ones_sb = singles.tile([128, 128], bf16)
        nc.vector.memset(ones_sb, 1.0 / S)
        eps_sb = singles.tile([128, 1], f32)
        nc.vector.memset(eps_sb, LN_EPS)

        u_col = singles.tile([128, NW * 32], f32)   # [t][k4][b8] per col
        st = singles.tile([128, 8192], bf16)        # [b8][k4][t256]

        def part(t):
            return t[:].ap[0]  # partition AP entry of a tile

        def vw(tile, off, dims):
            a = tile[:]
            return AP(tensor=a.tensor, offset=a.offset + off,
                      ap=[part(tile)] + [list(d) for d in dims])

        # ---- pre-pass: u'[t, k, b] = (x_b @ W_u + b_u) in column form ----
        xall = singles.tile([128, 64, NW], bf16)
        for b in range(B):
            eng = nc.sync if (b % 2 == 0) else nc.gpsimd
            eng.dma_start(out=xall[:, b * 8:(b + 1) * 8, :],
                          in_=xt[b].rearrange("k p t -> p k t"))
        for b in range(B):
            for m in range(4):
                ps = psum_big.tile([128, NW], f32)
                for k in range(8):
                    nc.tensor.matmul(
                        ps, wu_sb[:, k, m, :], xall[:, b * 8 + k, :],
                        start=(k == 0), stop=(k == 7)
                    )
                uout = vw(u_col, m * 8 + b, [[32, NW]])
                if (b * 4 + m) % 2 == 0:
                    nc.vector.tensor_scalar(
                        out=uout, in0=ps, scalar1=bu_sb[:, m:m + 1], scalar2=None,
                        op0=ALU.add)
                else:
                    nc.scalar.activation(
                        out=uout, in_=ps, func=AF.Identity,
                        bias=bu_sb[:, m:m + 1], scale=1.0)

        # ---- scan: 2 gangs x 3 chains ----
        # zs cols: s*96 + k*24 + ch*8 + b   (s=0: z, s=1: z^2), bf16
        # wt cols: k*24 + ch*8 + b (centered state w), bf16
        # abred cols: s*24 + ch*8 + b (mu | msq), f32
        def tg(shape, dt, nm):
            return [[singles.tile(shape, dt, name=f"{nm}{g}{p}") for p in range(2)]
                    for g in range(2)]

        zs = tg([128, 192], bf16, "zs")
        wt = tg([128, 96], bf16, "wt")
        abred = tg([128, 48], f32, "abred")
        mu2 = tg([128, 24], f32, "mu2")
        nv = tg([128, 24], f32, "nv")
        rr = tg([128, 24], f32, "rr")
        t1 = tg([128, 96], f32, "t1")
        zf1 = singles.tile([128, 32], f32)
        zf2 = singles.tile([128, 32], f32)

        KCB = [[24, 4], [8, 3], [1, 8]]          # (k, ch, b) over a [*,96] tile
        cneg_kcb = vw(cneg_sb, 0, [[1, 4], [0, 3], [0, 8]])

        def un_kcb(g, rel0, nch=3, ch0=0):
            # u' ganged view, dims (k, ch, b); chain stride 86 steps
            return vw(u_col, rel0 * 32 + ch0 * 86 * 32,
                      [[8, 4], [86 * 32, nch], [1, 8]])

        def rr_kcb(g, p, nch=3, ch0=0):
            return vw(rr[g][p], ch0 * 8, [[0, 4], [8, nch], [1, 8]])

        # gang init: z = u'[bases] + cneg; zsq = z^2
        for g in range(2):
            nc.vector.tensor_add(vw(zs[g][0], 0, KCB), un_kcb(g, BASES[g][0]),
                                 cneg_kcb)
            nc.scalar.activation(out=zs[g][0][:, 96:192], in_=zs[g][0][:, 0:96],
                                 func=AF.Square, bias=0.0, scale=1.0)

        def stage1(g, j):
            p = j % 2
            ab = psum_ab.tile([128, 192], f32)
            nc.tensor.matmul(ab, ones_sb, zs[g][p][:], start=True, stop=True,
                             skip_group_check=True)
            # mu | msq: reduce over k (two 4D-view reduces)
            for s in range(2):
                nc.vector.tensor_reduce(
                    out=abred[g][p][:, s * 24:(s + 1) * 24],
                    in_=vw(ab, s * 96, [[8, 3], [1, 8], [24, 4]]),
                    axis=mybir.AxisListType.X, op=ALU.add,
                )
            # w = z - mu (centered, bf16)
            nc.vector.tensor_sub(vw(wt[g][p], 0, KCB), vw(zs[g][p], 0, KCB),
                                 vw(abred[g][p], 0, [[0, 4], [8, 3], [1, 8]]))
            nc.gpsimd.tensor_mul(mu2[g][p][:], abred[g][p][:, 0:24],
                                 abred[g][p][:, 0:24])
            nc.gpsimd.tensor_sub(nv[g][p][:], mu2[g][p][:], abred[g][p][:, 24:48])
            nc.scalar.activation(out=rr[g][p][:], in_=nv[g][p][:],
                                 func=AF.Abs_reciprocal_sqrt,
                                 bias=eps_sb, scale=-1.0)
            return ab

        def stage2(g, j):
            p = j % 2
            q = 1 - p
            # chains still needing z_next at this step
            nch = sum(1 for c in range(3) if j < CS[g][c] - 1)
            # whitened states st[slot] = rr * w for output steps
            if j >= WARM:
                nst = sum(1 for c in range(3) if j < CS[g][c])
                slot0 = BASES[g][0] + (j - WARM)
                nc.gpsimd.tensor_mul(
                    vw(st, slot0, [[256, 4], [86, nst], [1024, 8]]),
                    vw(wt[g][p], 0, [[24, 4], [8, nst], [1, 8]]),
                    rr_kcb(g, p, nst))
            if nch == 0:
                return
            wp = psum_w.tile([128, 96], f32)
            for m in range(4):
                for k in range(4):
                    nc.tensor.matmul(
                        wp[:, m * 24:m * 24 + nch * 8], gt_sb[:, k, m, :],
                        wt[g][p][:, k * 24:k * 24 + nch * 8],
                        start=(k == 0), stop=(k == 3),
                    )
            kcb = [[24, 4], [8, nch], [1, 8]]
            nc.vector.tensor_mul(vw(t1[g][p], 0, kcb), vw(wp, 0, kcb),
                                 rr_kcb(g, p, nch))
            rel1 = BASES[g][0] + j + 1
            if g == 0 and j == WARM - 1:
                # splice exact z_{t=0} for core 0 on chain 0 (wmask zeroes it,
                # zinit = u'_0 + cneg); chains 1..2 take the normal path
                nc.vector.tensor_add(vw(zs[g][q], 8, [[24, 4], [8, 2], [1, 8]]),
                                     vw(t1[g][p], 8, [[24, 4], [8, 2], [1, 8]]),
                                     un_kcb(g, rel1, 2, 1))
                c0 = [[24, 4], [1, 8]]
                nc.vector.tensor_add(vw(zf1, 0, [[8, 4], [1, 8]]),
                                     vw(t1[g][p], 0, c0),
                                     vw(u_col, rel1 * 32, [[8, 4], [1, 8]]))
                nc.vector.tensor_mul(zf2[:], zf1[:], wmask_sb[:])
                nc.vector.tensor_add(vw(zs[g][q], 0, c0),
                                     vw(zf2, 0, [[8, 4], [1, 8]]),
                                     vw(zinit_sb, 0, [[8, 4], [1, 8]]))
                nc.scalar.activation(out=zs[g][q][:, 96:192],
                                     in_=zs[g][q][:, 0:96],
                                     func=AF.Square, bias=0.0, scale=1.0)
                return
            nc.vector.tensor_add(vw(zs[g][q], 0, kcb), vw(t1[g][p], 0, kcb),
                                 un_kcb(g, rel1, nch))
            nc.scalar.activation(out=vw(zs[g][q], 96, [[24, 4], [1, nch * 8]]),
                                 in_=vw(zs[g][q], 0, [[24, 4], [1, nch * 8]]),
                                 func=AF.Square, bias=0.0, scale=1.0)

        for j in range(59):
            if j == 1:
                # post-pass weights: load while the scan runs
                nc.sync.dma_start(out=wo_sb,
                                  in_=wo.rearrange("(k p) e -> p k e", p=128))
                nc.sync.dma_start(
                    out=bo_sb,
                    in_=AP(tensor=bo_ap.tensor, offset=bo_ap.offset,
                           ap=[[0, 128], [1, E]]),
                )
            for g in range(2):
                stage1(g, j)
            for g in range(2):
                stage2(g, j)

        # ---- post-pass: out = st @ W_o + b_out ----
        for b in range(B):
            for th in range(2):
                ob = opool.tile([128, E], f32)
                for ec in range(2):
                    ps = psum_big.tile([128, 512], f32)
                    for k in range(4):
                        sa = st[:]
                        lhsT = AP(tensor=sa.tensor,
                                  offset=sa.offset + b * 1024 + k * 256 + th * 128,
                                  ap=[part(st), [1, 128]])
                        nc.tensor.matmul(ps, lhsT, wo_sb[:, k, ec * 512:(ec + 1) * 512],
                                         start=(k == 0), stop=(k == 3))
                    nc.vector.tensor_add(ob[:, ec * 512:(ec + 1) * 512], ps,
                                         bo_sb[:, ec * 512:(ec + 1) * 512])
                nc.sync.dma_start(out=y[b, th * 128:(th + 1) * 128, :], in_=ob)

    nc.compile()
    return nc


def host_prep(inputs):
    """Fold parameters on host; returns (shared dict, per-core dicts)."""
    from ml_dtypes import bfloat16

    et = np.asarray(inputs["embedded_tokens"], np.float32)
    W_e2s = np.asarray(inputs["W_e2s"], np.float64)
    b_e2s = np.asarray(inputs["b_e2s"], np.float64)
    A = np.asarray(inputs["A"], np.float64)
    Bm = np.asarray(inputs["Bm"], np.float64)
    C = np.asarray(inputs["C"], np.float64)
    gamma = np.asarray(inputs["ln_gamma"], np.float64)
    beta = np.asarray(inputs["ln_beta"], np.float64)
    W_s2o = np.asarray(inputs["W_s2o"], np.float64)
    b_s2o = np.asarray(inputs["b_s2o"], np.float64)

    W_u = (W_e2s @ Bm).astype(np.float32)
    b_u = (b_e2s @ Bm + beta @ A).astype(np.float32)
    G = (gamma[:, None] * A).astype(np.float32)
    Gb = G.astype(bfloat16)
    W_o = ((gamma[:, None] * C) @ W_s2o).astype(np.float32)
    b_out = (beta @ C @ W_s2o + b_s2o).astype(np.float32)
    cneg = (-(beta @ A)).astype(np.float32)

    Wub = W_u.astype(bfloat16)
    wu_tiles = np.ascontiguousarray(
        Wub.reshape(8, 128, 4, 128).transpose(0, 2, 1, 3))
    gt_tiles = np.ascontiguousarray(
        Gb.reshape(4, 128, 4, 128).transpose(0, 2, 1, 3))

    shared = {
        "wu": wu_tiles,
        "gt": gt_tiles,
        "wo": np.ascontiguousarray(W_o.astype(bfloat16)),
        "buc": np.ascontiguousarray(b_u.reshape(4, 128).T),
        "cneg": np.ascontiguousarray(cneg.reshape(4, 128).T),
        "bo": np.ascontiguousarray(b_out.reshape(1, E)),
        "bo16": np.ascontiguousarray(b_out.reshape(1, E).astype(bfloat16)),
    }

    # zinit for core 0: u'_0 + cneg (device-matching bf16 x @ bf16 W_u)
    x0 = et[:, 0, :].astype(bfloat16).astype(np.float32)
    u0 = x0 @ Wub.astype(np.float32) + b_u + cneg          # [B, S]
    zinit0 = np.ascontiguousarray(
        u0.reshape(B, 4, 128).transpose(2, 1, 0).reshape(128, 32))
    z0 = np.zeros((128, 32), np.float32)
    m0 = np.zeros((128, 32), np.float32)
    m1 = np.ones((128, 32), np.float32)

    etb = et.astype(bfloat16)
    per_core = []
    for c in range(NCORES):
        t0 = c * WOUT
        lo = t0 - WARM
        xw = np.zeros((B, NW, E), bfloat16)
        pad = max(0, -lo)
        xw[:, pad:, :] = etb[:, max(0, lo):t0 + WOUT, :]
        xtc = np.ascontiguousarray(
            xw.transpose(0, 2, 1).reshape(B, 8, 128, NW))
        per_core.append({
            "xt": xtc,
            "zinit": zinit0 if c == 0 else z0,
            "wmask": m0 if c == 0 else m1,
        })
    return shared, per_core


def kernel(**inputs):
    key = "nc2"
    if key not in _CACHE:
        _CACHE[key] = build()
    nc = _CACHE[key]

    from concourse.bass_utils import run_bass_kernel_spmd

    shared, per_core = host_prep(inputs)
    in_maps = [dict(shared, **per_core[c]) for c in range(NCORES)]
    res = run_bass_kernel_spmd(nc, in_maps, core_ids=list(range(NCORES)))
    out = np.empty((B, T, E), np.float32)
    for c in range(NCORES):
        out[:, c * WOUT:(c + 1) * WOUT, :] = np.asarray(res.results[c]["y"], np.float32)
    return out
